# revision 16
# baseline (speedup 1.0000x reference)
"""Trainium2 Bass kernel for nn_CAFF_3100966388292.

Dual-stream (SAR/OPT) cross-attention fusion net:
  theta/phi/g 1x1-conv projections on both streams, per-sample NxN attention
  maps fused elementwise, both value streams attended, product taken, output
  1x1-conv + residual + channel-mean pool + linear head.

Strategy (pure data parallel, 4 samples per core on 8 cores):
  * Transposed-logits layout: L^T(m,n) = phi^T theta on the PE, exp on ACT,
    S = Ex*Ey on DVE, all tiles (m-part, n-free).
  * Softmax denominators WITHOUT full ones-matmul streaming: DVE 3-level
    tree reduces the six 128-row chunks of E into Esum, then 6 F=1 matmuls
    per stream (lhsT = Esum chunk, rhs = ones column) drop Z directly into
    COLUMN layout [128n, 1] - no [1,N] row ops anywhere.
  * U-matmuls swapped (lhsT = S chunk, rhs = [Gx | Gy*wbar]) so attended
    values land [n-part, ci-free]; wbar (the algebraically-collapsed W
    projection + channel-mean) is folded into the OPT g-weights on host, so
    qraw(n) = sum_ci Ux*Uy' comes straight out of one fused DVE
    tensor_tensor_reduce per n-chunk. The whole per-sample fixup
    (1/(Zx*Zy)^2 * qraw + residual) runs in column space [128, 6].
  * Residual colsums ride the bf16 g-projection as a 257th weight column
    (fp8 inputs are NOT precise enough for the residual path - measured).
  * theta/phi projections + logits in fp8 DoubleRow (2 k-chunks/instr).
  * PSUM-drain copies rotate over ACT/DVE/GpSimd so no single co-engine
    throttles the PE; PE warm-up matmuls raise the pstate before real work.
"""

import sys
import types

import ml_dtypes
import numpy as np

# The agent image's antenv package lacks axon_hooks; register the equivalent
# NTFF hook so run_bass_kernel_spmd(trace=True) works if ever requested.
try:  # pragma: no cover
    import antenv.axon_hooks  # noqa: F401
except ImportError:
    try:
        from trn_agent_boot.trn_boot import _ntff_profile_via_ctypes

        _hook = _ntff_profile_via_ctypes("/opt/axon/libaxon_pjrt.so")
        _mod = types.ModuleType("antenv.axon_hooks")
        _mod.get_axon_ntff_profile_hook = lambda: _hook
        _mod.set_axon_ntff_profile_hook = lambda h: None
        sys.modules["antenv.axon_hooks"] = _mod
    except Exception:
        pass

import concourse.bass as bass
import concourse.tile as tile
from concourse import bacc, mybir
from concourse.bass_utils import run_bass_kernel_spmd

F32 = mybir.dt.float32
BF16 = mybir.dt.bfloat16
FP8 = mybir.dt.float8e4
EXP_SHIFT = -12.0  # constant logit shift before exp; cancels exactly in the math

B, C, CI, N, HOUT = 32, 512, 256, 768, 256
NCORES = 8
BPC = B // NCORES  # samples per core
KC = C // 128  # 4 k-chunks over channels
MC = N // 128  # 6 chunks over positions
CIC = CI // 128  # 2 chunks over inner channels
# free-dim split of N into PSUM-bank-legal matmul halves
NH = ((0, 512), (512, 256))

_cached = {}

# Feature toggles for hw bisection (all-on is the intended design).
import os as _os
FEAT = {
    "warmup": _os.environ.get("K_WARMUP", "1") == "1",
    "gp": _os.environ.get("K_GP", "1") == "1",       # gpsimd S-mult/tree
    "ttr": _os.environ.get("K_TTR", "1") == "1",     # fused qraw reduce
    "zmm": _os.environ.get("K_ZMM", "1") == "1",     # F=1 Z-column matmuls
}


def _pack(a):
    """(R, F) host array -> (128, R//128 * F) partition-major bf16."""
    a = np.asarray(a, dtype=np.float32)
    r, f = a.shape
    k = r // 128
    return np.ascontiguousarray(
        a.reshape(k, 128, f).transpose(1, 0, 2).reshape(128, k * f)
    ).astype(ml_dtypes.bfloat16)


def _build(has_gb_x, has_gb_y, has_hb, has_tb, _feat=None):
    nc = bacc.Bacc("TRN2", target_bir_lowering=False, debug=False)
    AF = mybir.ActivationFunctionType
    MULT = mybir.AluOpType.mult
    ADD = mybir.AluOpType.add

    def mm(out, lhsT, rhs, start, stop):
        nc.tensor.matmul(out, lhsT, rhs, start=start, stop=stop)

    def mmdr(out, lhsT, rhs, start, stop):
        nc.tensor.matmul(out, lhsT, rhs, start=start, stop=stop,
                         perf_mode=mybir.MatmulPerfMode.DoubleRow)

    # inputs host-packed to (BPC, 128, KC*N) partition-major
    d_sar = nc.dram_tensor("sar", [BPC, 128, KC * N], BF16, kind="ExternalInput")
    d_opt = nc.dram_tensor("opt", [BPC, 128, KC * N], BF16, kind="ExternalInput")
    d_sar8 = nc.dram_tensor("sar8", [BPC, 128, KC * N], FP8, kind="ExternalInput")
    d_opt8 = nc.dram_tensor("opt8", [BPC, 128, KC * N], FP8, kind="ExternalInput")
    # host-pretransposed + packed projection weights
    d_w = {}
    for nm in ("wt_tx", "wt_px", "wt_ty", "wt_py"):
        d_w[nm] = nc.dram_tensor(nm, [128, KC * CI], FP8, kind="ExternalInput")
    for nm in ("wt_gx", "wt_gy"):  # g weights carry a gamma/C ones column
        d_w[nm] = nc.dram_tensor(nm, [128, KC * (CI + 1)], BF16,
                                 kind="ExternalInput")
    d_hwT = nc.dram_tensor("hwT", [128, MC * HOUT], BF16, kind="ExternalInput")
    d_tb = {}
    if has_tb:  # theta/phi per-partition bias columns (CI,), fp32 (ACT bias)
        d_tb = {
            nm: nc.dram_tensor(nm, [CI], F32, kind="ExternalInput")
            for nm in ("b_tx", "b_px", "b_ty", "b_py")
        }
    d_ones = nc.dram_tensor("ones", [128, 256], BF16, kind="ExternalInput")
    need_onesr = has_gb_x or has_gb_y or has_hb
    if need_onesr:
        d_onesr = nc.dram_tensor("ones_row", [1, 128], BF16, kind="ExternalInput")
    d_expb = nc.dram_tensor("expb", [128, 1], F32, kind="ExternalInput")
    d_gb = {}
    if has_gb_x:
        d_gb["x"] = nc.dram_tensor("gb_x", [1, CI], BF16, kind="ExternalInput")
    if has_gb_y:
        d_gb["y"] = nc.dram_tensor("gb_y", [1, CI], BF16, kind="ExternalInput")
    if has_hb:
        d_hb = nc.dram_tensor("hb", [1, HOUT], BF16, kind="ExternalInput")
    d_out = nc.dram_tensor("out", [BPC, HOUT], F32, kind="ExternalOutput")

    with tile.TileContext(nc) as tc, \
            tc.tile_pool(name="wts", bufs=1) as wts, \
            tc.tile_pool(name="inp", bufs=1) as inp, \
            tc.tile_pool(name="proj", bufs=2) as proj, \
            tc.tile_pool(name="gp", bufs=2) as gpool, \
            tc.tile_pool(name="att", bufs=2) as attp, \
            tc.tile_pool(name="scr", bufs=3) as scr, \
            tc.tile_pool(name="rows", bufs=1) as rows, \
            tc.tile_pool(name="psA", bufs=2, space="PSUM") as psA, \
            tc.tile_pool(name="psG", bufs=2, space="PSUM") as psG, \
            tc.tile_pool(name="psU", bufs=2, space="PSUM") as psU:

        # ---- DMAs in strict first-use order (queues are FIFO) ----
        ones = wts.tile([128, 256], BF16, tag="ones", name="ones")
        nc.sync.dma_start(ones[:], d_ones.ap())

        def load_w(nm, cols=CI, dt_=None):
            t = wts.tile([128, KC, cols],
                         dt_ or (FP8 if nm[3] in "tp" else BF16), tag=nm, name=nm)
            nc.sync.dma_start(t[:], d_w[nm].ap().rearrange("p (k f) -> p k f", k=KC))
            return t

        w_sb = {}
        # first weight + first input gate the kernel start
        t = wts.tile([128, KC, CI], FP8, tag="wt_tx", name="wt_tx")
        w_sb["wt_tx"] = t
        nc.sync.dma_start(t[:, 0:2, :],
                          d_w["wt_tx"].ap()[:, :2 * CI].rearrange(
                              "p (k f) -> p k f", k=2))
        x8 = inp.tile([128, BPC, KC, N], FP8, tag="x8", name="x8")
        y8 = inp.tile([128, BPC, KC, N], FP8, tag="y8", name="y8")
        x_t = inp.tile([128, BPC, KC, N], BF16, tag="x_t", name="x_t")
        y_t = inp.tile([128, BPC, KC, N], BF16, tag="y_t", name="y_t")
        nc.sync.dma_start(x8[:, 0], d_sar8[0].rearrange("p (k n) -> p k n", k=KC))
        nc.sync.dma_start(t[:, 2:, :],
                          d_w["wt_tx"].ap()[:, 2 * CI:].rearrange(
                              "p (k f) -> p k f", k=KC - 2))
        w_sb["wt_px"] = load_w("wt_px")
        nc.sync.dma_start(x_t[:, 0], d_sar[0].rearrange("p (k n) -> p k n", k=KC))
        w_sb["wt_gx"] = load_w("wt_gx", CI + 1)
        w_sb["wt_ty"] = load_w("wt_ty")
        w_sb["wt_py"] = load_w("wt_py")
        nc.sync.dma_start(y8[:, 0], d_opt8[0].rearrange("p (k n) -> p k n", k=KC))
        w_sb["wt_gy"] = load_w("wt_gy", CI + 1)
        nc.sync.dma_start(y_t[:, 0], d_opt[0].rearrange("p (k n) -> p k n", k=KC))
        nc.sync.dma_start(x8[:, 1], d_sar8[1].rearrange("p (k n) -> p k n", k=KC))
        nc.sync.dma_start(y8[:, 1], d_opt8[1].rearrange("p (k n) -> p k n", k=KC))
        expb = wts.tile([128, 1], F32, tag="expb", name="expb")
        nc.sync.dma_start(expb[:], d_expb.ap())
        tb_sb = {}
        for nm, d in d_tb.items():
            tt = wts.tile([128, CIC], F32, tag=nm, name=nm)
            nc.sync.dma_start(tt[:], d.ap().rearrange("(k p) -> p k", p=128))
            tb_sb[nm] = tt
        for s in range(1, BPC):
            if s > 1:
                nc.sync.dma_start(x8[:, s],
                                  d_sar8[s].rearrange("p (k n) -> p k n", k=KC))
                nc.sync.dma_start(y8[:, s],
                                  d_opt8[s].rearrange("p (k n) -> p k n", k=KC))
            nc.sync.dma_start(x_t[:, s], d_sar[s].rearrange("p (k n) -> p k n", k=KC))
            nc.sync.dma_start(y_t[:, s], d_opt[s].rearrange("p (k n) -> p k n", k=KC))
        hwT = wts.tile([128, MC, HOUT], BF16, tag="hwT", name="hwT")
        nc.sync.dma_start(hwT[:], d_hwT.ap().rearrange("p (k f) -> p k f", k=MC))
        if need_onesr:
            ones_row = wts.tile([1, 128], BF16, tag="ones_row", name="ones_row")
            nc.sync.dma_start(ones_row[:], d_onesr.ap())
        gb_sb = {}
        for st, d in d_gb.items():
            tt = wts.tile([1, CI], BF16, tag=f"gb_{st}", name=f"gb_{st}")
            nc.sync.dma_start(tt[:], d.ap())
            gb_sb[st] = tt
        if has_hb:
            hb = wts.tile([1, HOUT], BF16, tag="hb", name="hb")
            nc.sync.dma_start(hb[:], d_hb.ap())

        in8 = {"x": x8, "y": y8}
        int_ = {"x": x_t, "y": y_t}
        pooledT = rows.tile([128, MC, BPC], BF16, tag="pooledT", name="pooledT")

        # ---- PE warm-up: ramp the pstate while input DMAs land ----
        for i in range(12 if FEAT["warmup"] else 0):
            pw = psU.tile([128, 512], F32, tag="u", name="pw")
            mm(pw[:, 0:256], ones[:, 0:128], ones[:], True, True)

        # PSUM-drain copy with round-robin engine assignment.
        # GpSimd cannot access PSUM, so drains rotate over ACT/DVE only.
        def copy_out(dst, src, eng):
            if eng == "act":
                nc.scalar.copy(dst, src)
            else:
                nc.vector.tensor_copy(dst, src)

        def emit_tp_chunk(s, ch, defer):
            """One theta/phi projection chunk (128 ci-rows x N) of sample s.
            Returns (dst, cic, pt) when the PSUM drain is deferred."""
            st, pr, cic, dst, eng = ch
            w = w_sb[f"wt_{pr}{st}"]
            pt = psA.tile([128, N], F32, tag="ps", name="ps")
            for kp in range(KC // 2):
                for o, f in NH:
                    mmdr(pt[:, o:o + f],
                         w[:, 2 * kp:2 * kp + 2, cic * 128:(cic + 1) * 128],
                         in8[st][:, s, 2 * kp:2 * kp + 2, o:o + f],
                         kp == 0, kp == KC // 2 - 1)
            if has_tb:
                nc.scalar.activation(dst[:, cic, :], pt[:], AF.Identity,
                                     bias=tb_sb[f"b_{pr}{st}"][:, cic:cic + 1])
                return None
            if defer:
                return (dst, cic, pt)
            copy_out(dst[:, cic, :], pt[:], eng)
            return None

        def tp_list(s, engs):
            pj = {}
            lst = []
            for i, (st, pr) in enumerate(
                    ((st, pr) for st in ("x", "y") for pr in ("t", "p"))):
                dst = proj.tile([128, CIC, N], FP8, tag=f"pj_{pr}{st}",
                                name=f"pj_{pr}{st}")
                pj[pr + st] = dst
                for cic in range(CIC):
                    lst.append((st, pr, cic, dst, engs[(2 * i + cic) % len(engs)]))
            return pj, lst

        def emit_g_chunk(s, sti, st, mc_, gXY, rscol, eng):
            """One g-projection column chunk; residual colsum rides col CI."""
            w = w_sb[f"wt_g{st}"]
            has_b = st in gb_sb
            pt = psG.tile([128, 512], F32, tag="g", name="gps")
            for k in range(KC):
                mm(pt[:, 0:CI + 1],
                   int_[st][:, s, k, mc_ * 128:(mc_ + 1) * 128],
                   w[:, k, :], k == 0, (k == KC - 1) and not has_b)
            if has_b:
                mm(pt[:, 0:CI], ones_row[:], gb_sb[st][:], False, True)
            copy_out(gXY[:, mc_, sti * CI:(sti + 1) * CI], pt[:, 0:CI], eng)
            if st == "x":
                nc.scalar.copy(rscol[:, mc_:mc_ + 1], pt[:, CI:CI + 1])
            else:
                nc.vector.tensor_add(rscol[:, mc_:mc_ + 1],
                                     rscol[:, mc_:mc_ + 1], pt[:, CI:CI + 1])

        def new_g_tiles():
            gXY = gpool.tile([128, MC, 2 * CI], BF16, tag="gXY", name="gXY")
            rscol = scr.tile([128, MC], F32, tag="rscol", name="rscol")
            return gXY, rscol

        # ---- preamble: theta/phi of sample 0 (copies split ACT/DVE) ----
        pj, lst0 = tp_list(0, ("act", "dve"))
        for ch in lst0:
            emit_tp_chunk(0, ch, False)

        for s in range(BPC):
            nxt = s + 1 < BPC
            # fillers interleaved into the logits phase: g(s) first (it gates
            # this iteration's U), mc-major so early chunks land before the
            # laddered U needs them, then theta/phi(s+1) whose last two PSUM
            # drains are deferred (on ACT) into the U phase.
            fillers = []
            if s == 0 or s == BPC - 1:
                gXY, rscol = new_g_tiles()
                for mc_ in range(MC):
                    for sti, st in enumerate(("x", "y")):
                        fillers.append(("g", (s, sti, st, mc_, gXY, rscol)))
            if nxt:
                pj_next, tpl = tp_list(s + 1, ("dve",))
                fillers += [("tp", ch) for ch in tpl]
            else:
                pj_next = None
            nf = len(fillers)

            # -- transposed logits + exp + fused map, fillers interleaved --
            E = {st: attp.tile([128, MC, N], BF16, tag=f"E{st}", name=f"E{st}")
                 for st in ("x", "y")}
            S = attp.tile([128, MC, N], BF16, tag="S", name="S")
            deferred = []
            fi = 0
            for mc_ in range(MC):
                for st in ("x", "y"):
                    pt = psA.tile([128, N], F32, tag="ps", name="ps")
                    for o, f in NH:
                        mmdr(pt[:, o:o + f],
                             pj["p" + st][:, :, mc_ * 128:(mc_ + 1) * 128],
                             pj["t" + st][:, :, o:o + f], True, True)
                    nc.scalar.activation(E[st][:, mc_, :], pt[:], AF.Exp,
                                         bias=expb[:])
                # fused map on SBUF operands: early chunks on idle GpSimd
                # (latency hidden behind later exps), tail chunks on DVE;
                # the last sample is tail-latency-critical: all DVE
                seng = nc.gpsimd if (mc_ < 3 and FEAT["gp"] and nxt) \
                    else nc.vector
                seng.tensor_mul(S[:, mc_, :], E["x"][:, mc_, :],
                                E["y"][:, mc_, :])
                hi = nf * (mc_ + 1) // MC
                while fi < hi:
                    kind, data = fillers[fi]
                    fi += 1
                    if kind == "g":
                        emit_g_chunk(*data, eng=("act", "dve")[fi % 2])
                    else:
                        d = emit_tp_chunk(s + 1, data, fi > nf - 2)
                        if d is not None:
                            deferred.append(d)

            # -- deferred theta/phi drains (ACT has slack in the U phase) --
            for dst, cic, ptd in deferred:
                nc.scalar.copy(dst[:, cic, :], ptd[:])

            # -- Esum trees, pairwise so early adds overlap the exp phase
            # and the post-E5 critical path is two adds. Early pairs ride the
            # idle GpSimd except on the tail-critical last sample. --
            Es = {}
            geng = nc.gpsimd if (FEAT["gp"] and nxt) else nc.vector
            for st in ("x", "y"):
                P01 = scr.tile([128, N], BF16, tag="Pa", name="Pa")
                geng.tensor_add(P01[:], E[st][:, 0, :], E[st][:, 1, :])
                P23 = scr.tile([128, N], BF16, tag="Pb", name="Pb")
                nc.vector.tensor_add(P23[:], E[st][:, 2, :], E[st][:, 3, :])
                P45 = scr.tile([128, N], BF16, tag="Pc", name="Pc")
                nc.vector.tensor_add(P45[:], E[st][:, 4, :], E[st][:, 5, :])
                Q = scr.tile([128, N], BF16, tag="Q", name="Q")
                nc.vector.tensor_add(Q[:], P01[:], P23[:])
                Et = scr.tile([128, N], BF16, tag=f"Es{st}", name=f"Es{st}")
                nc.vector.tensor_add(Et[:], Q[:], P45[:])
                Es[st] = Et

            # -- attention apply, both streams per instruction; fused qraw --
            # Last sample has no theta/phi filler to absorb the exp drain, so
            # ladder the accumulation (2 live PSUM tiles) to start U earlier.
            qcol = scr.tile([128, MC], F32, tag="qcol", name="qcol")
            if s == BPC - 1:
                seq = []
                for n in range(MC + 1):
                    if n < MC:
                        seq.append((n, 0))
                    if n >= 1 and n < MC:
                        seq.append((n - 1, 1))
                    if n >= 1:
                        seq.append((n - 1, 2))
            else:
                seq = [(n, h) for n in range(MC) for h in (0, 1, 2)]
            pus = {}
            for nc_, third in seq:
                if third == 0:
                    pus[nc_] = psU.tile([128, 512], F32, tag="u", name="pu")
                pu = pus[nc_]
                for mc_ in range(2 * third, 2 * third + 2):
                    mm(pu[:], S[:, mc_, nc_ * 128:(nc_ + 1) * 128],
                       gXY[:, mc_, :], mc_ == 0, mc_ == MC - 1)
                if third == 2:
                    ux = scr.tile([128, CI], BF16, tag="ux", name="ux")
                    nc.scalar.copy(ux[:], pu[:, 0:CI])
                    yv = scr.tile([128, CI], BF16, tag="yv", name="yv")
                    nc.vector.tensor_mul(yv[:], ux[:], pu[:, CI:2 * CI])
                    nc.vector.tensor_reduce(
                        qcol[:, nc_:nc_ + 1], yv[:],
                        axis=mybir.AxisListType.X, op=ADD)
                    del pus[nc_]  # noqa

            # -- Z columns: F=1 matmuls on Esum chunks --
            zs = scr.tile([128, 12], F32, tag="zs", name="zs")
            if FEAT["zmm"]:
                zt = psU.tile([128, 512], F32, tag="u", name="zt")
                for sti, st in enumerate(("x", "y")):
                    for nc_ in range(MC):
                        mm(zt[:, 6 * sti + nc_:6 * sti + nc_ + 1],
                           Es[st][:, nc_ * 128:(nc_ + 1) * 128],
                           ones[:, 0:1], True, True)
                nc.scalar.copy(zs[:], zt[:, 0:12])
            else:
                nc.vector.memset(zs[:], 1.0)

            # -- column-space fixup: pooled = qraw/(Zx*Zy)^2 + rs --
            p1 = scr.tile([128, MC], F32, tag="p1", name="p1")
            nc.vector.tensor_mul(p1[:], zs[:, 0:6], zs[:, 6:12])
            p2 = scr.tile([128, MC], F32, tag="p2", name="p2")
            nc.vector.reciprocal(p2[:], p1[:])
            p3 = scr.tile([128, MC], F32, tag="p3", name="p3")
            nc.vector.tensor_mul(p3[:], p2[:], p2[:])
            p4 = scr.tile([128, MC], F32, tag="p4", name="p4")
            nc.vector.tensor_mul(p4[:], p3[:], qcol[:])
            nc.vector.tensor_add(pooledT[:, :, s], p4[:], rscol[:])

            # -- g projections of next sample (middle iterations only; the
            # last sample's g is its own phase-1 filler) --
            if nxt and s + 1 < BPC - 1:
                gXY, rscol = new_g_tiles()
                gi = 0
                for sti, st in enumerate(("x", "y")):
                    for mc_ in range(MC):
                        emit_g_chunk(s + 1, sti, st, mc_, gXY, rscol,
                                     ("act", "dve")[gi % 2])
                        gi += 1
            pj = pj_next

        # ---- head ----
        pt = psU.tile([128, 512], F32, tag="u", name="head_ps")
        for j in range(MC):
            mm(pt[0:BPC, 0:HOUT], pooledT[:, j, :], hwT[:, j, :],
               j == 0, (j == MC - 1) and not has_hb)
        if has_hb:
            mm(pt[0:BPC, 0:HOUT], ones_row[:, :BPC], hb[:], False, True)
        out_sb = rows.tile([BPC, HOUT], F32, tag="out_sb", name="out_sb")
        nc.scalar.copy(out_sb[:], pt[0:BPC, 0:HOUT])
        nc.sync.dma_start(d_out[:], out_sb[:])

    nc.compile()
    return nc


def _prepare(inputs):
    f = lambda k: np.ascontiguousarray(np.asarray(inputs[k], dtype=np.float32))
    bf = lambda a: np.ascontiguousarray(np.asarray(a, dtype=ml_dtypes.bfloat16))
    sar, opt = f("sar"), f("opt")
    ga = float(np.asarray(inputs["gamma_att"]).reshape(-1)[0])
    go = float(np.asarray(inputs["gamma_opt"]).reshape(-1)[0])
    gs = float(np.asarray(inputs["gamma_sar"]).reshape(-1)[0])
    W_w, W_b = f("W_w"), f("W_b")
    head_w, head_b = f("head_w"), f("head_b")

    wbar = (ga / C) * W_w.sum(axis=0)  # (CI,)
    bbar = (ga / C) * float(W_b.sum())
    # fold the pooled-constant through the head: out += bbar * head_w.sum(1)
    hb_eff = head_b + bbar * head_w.sum(axis=1)  # (HOUT,)

    gb_x = f("g_sar_b")
    gb_y = f("g_opt_b") * wbar  # wbar folded into the OPT value stream
    has_gb_x = bool(np.any(gb_x))
    has_gb_y = bool(np.any(gb_y))
    has_hb = bool(np.any(hb_eff))
    has_tb = any(bool(np.any(f(nm))) for nm in
                 ("theta_sar_b", "phi_sar_b", "theta_opt_b", "phi_opt_b"))

    key = (has_gb_x, has_gb_y, has_hb, has_tb, tuple(sorted(FEAT.items())))
    if key not in _cached:
        _cached[key] = _build(*key[:4])
    nc = _cached[key]

    # pack inputs: (B, C, N) -> per-core (BPC, 128, KC*N) partition-major
    def pack_in(a):
        a = a.reshape(B, KC, 128, N).transpose(0, 2, 1, 3).reshape(B, 128, KC * N)
        return np.ascontiguousarray(a).astype(ml_dtypes.bfloat16)

    sar_p, opt_p = pack_in(sar), pack_in(opt)

    p8 = lambda a: _pack(a).astype(ml_dtypes.float8_e4m3fn)
    common = {
        "wt_tx": p8(f("theta_sar_w").T),
        "wt_px": p8(f("phi_sar_w").T),
        "wt_ty": p8(f("theta_opt_w").T),
        "wt_py": p8(f("phi_opt_w").T),
        "wt_gx": _pack(np.concatenate(
            [f("g_sar_w").T, np.full((C, 1), gs / C, np.float32)], axis=1)),
        "wt_gy": _pack(np.concatenate(
            [(f("g_opt_w") * wbar[:, None]).T,
             np.full((C, 1), go / C, np.float32)], axis=1)),
        "hwT": _pack(head_w.T),
        "ones": np.ones((128, 256), ml_dtypes.bfloat16),
        "expb": np.full((128, 1), EXP_SHIFT, np.float32),
    }
    if has_tb:
        common.update({
            "b_tx": f("theta_sar_b"), "b_px": f("phi_sar_b"),
            "b_ty": f("theta_opt_b"), "b_py": f("phi_opt_b"),
        })
    if has_gb_x or has_gb_y or has_hb:
        common["ones_row"] = np.ones((1, 128), ml_dtypes.bfloat16)
    if has_gb_x:
        common["gb_x"] = bf(gb_x.reshape(1, CI))
    if has_gb_y:
        common["gb_y"] = bf(gb_y.reshape(1, CI))
    if has_hb:
        common["hb"] = bf(hb_eff.reshape(1, HOUT))

    in_maps = []
    for c in range(NCORES):
        m = dict(common)
        m["sar"] = np.ascontiguousarray(sar_p[c * BPC:(c + 1) * BPC])
        m["opt"] = np.ascontiguousarray(opt_p[c * BPC:(c + 1) * BPC])
        m["sar8"] = m["sar"].astype(ml_dtypes.float8_e4m3fn)
        m["opt8"] = m["opt"].astype(ml_dtypes.float8_e4m3fn)
        in_maps.append(m)
    return nc, in_maps


def kernel(**inputs):
    nc, in_maps = _prepare(inputs)
    res = run_bass_kernel_spmd(nc, in_maps, core_ids=list(range(NCORES)))
    return np.concatenate([res.results[c]["out"] for c in range(NCORES)], axis=0)


if __name__ == "__main__":
    rng = np.random.default_rng(0)
    ins = {
        "sar": rng.standard_normal((B, C, N), dtype=np.float32),
        "opt": rng.standard_normal((B, C, N), dtype=np.float32),
    }
    for nm in ("g_sar", "g_opt", "theta_sar", "theta_opt", "phi_sar", "phi_opt"):
        ins[nm + "_w"] = 0.02 * rng.standard_normal((CI, C), dtype=np.float32)
        ins[nm + "_b"] = np.zeros((CI,), np.float32)
    ins["W_w"] = 0.02 * rng.standard_normal((C, CI), dtype=np.float32)
    ins["W_b"] = np.zeros((C,), np.float32)
    ins["head_w"] = 0.02 * rng.standard_normal((HOUT, N), dtype=np.float32)
    ins["head_b"] = np.zeros((HOUT,), np.float32)
    ins["gamma_sar"] = np.asarray([0.3], np.float32)
    ins["gamma_opt"] = np.asarray([1.0], np.float32)
    ins["gamma_att"] = np.asarray([1.0], np.float32)
    out = kernel(**ins)
    print(out.shape, out.dtype, np.abs(out).mean())


# revision 19
# speedup vs baseline: 1.0422x; 1.0422x over previous
"""Trainium2 Bass kernel for nn_CAFF_3100966388292.

Dual-stream (SAR/OPT) cross-attention fusion net:
  theta/phi/g 1x1-conv projections on both streams, per-sample NxN attention
  maps fused elementwise, both value streams attended, product taken, output
  1x1-conv + residual + channel-mean pool + linear head.

Strategy (pure data parallel, 4 samples per core on 8 cores):
  * Transposed-logits layout: L^T(m,n) = phi^T theta on the PE, exp on ACT,
    S = Ex*Ey on DVE, all tiles (m-part, n-free).
  * Softmax denominators WITHOUT full ones-matmul streaming: DVE 3-level
    tree reduces the six 128-row chunks of E into Esum, then 6 F=1 matmuls
    per stream (lhsT = Esum chunk, rhs = ones column) drop Z directly into
    COLUMN layout [128n, 1] - no [1,N] row ops anywhere.
  * U-matmuls swapped (lhsT = S chunk, rhs = [Gx | Gy*wbar]) so attended
    values land [n-part, ci-free]; wbar (the algebraically-collapsed W
    projection + channel-mean) is folded into the OPT g-weights on host, so
    qraw(n) = sum_ci Ux*Uy' comes straight out of one fused DVE
    tensor_tensor_reduce per n-chunk. The whole per-sample fixup
    (1/(Zx*Zy)^2 * qraw + residual) runs in column space [128, 6].
  * Residual colsums ride the bf16 g-projection as a 257th weight column
    (fp8 inputs are NOT precise enough for the residual path - measured).
  * theta/phi projections + logits in fp8 DoubleRow (2 k-chunks/instr).
  * PSUM-drain copies rotate over ACT/DVE/GpSimd so no single co-engine
    throttles the PE; PE warm-up matmuls raise the pstate before real work.
"""

import sys
import types

import ml_dtypes
import numpy as np

# The agent image's antenv package lacks axon_hooks; register the equivalent
# NTFF hook so run_bass_kernel_spmd(trace=True) works if ever requested.
try:  # pragma: no cover
    import antenv.axon_hooks  # noqa: F401
except ImportError:
    try:
        from trn_agent_boot.trn_boot import _ntff_profile_via_ctypes

        _hook = _ntff_profile_via_ctypes("/opt/axon/libaxon_pjrt.so")
        _mod = types.ModuleType("antenv.axon_hooks")
        _mod.get_axon_ntff_profile_hook = lambda: _hook
        _mod.set_axon_ntff_profile_hook = lambda h: None
        sys.modules["antenv.axon_hooks"] = _mod
    except Exception:
        pass

import concourse.bass as bass
import concourse.tile as tile
from concourse import bacc, mybir
from concourse.bass_utils import run_bass_kernel_spmd

F32 = mybir.dt.float32
BF16 = mybir.dt.bfloat16
FP8 = mybir.dt.float8e4
EXP_SHIFT = -12.0  # constant logit shift before exp; cancels exactly in the math

B, C, CI, N, HOUT = 32, 512, 256, 768, 256
NCORES = 8
BPC = B // NCORES  # samples per core
KC = C // 128  # 4 k-chunks over channels
MC = N // 128  # 6 chunks over positions
CIC = CI // 128  # 2 chunks over inner channels
# free-dim split of N into PSUM-bank-legal matmul halves
NH = ((0, 512), (512, 256))

_cached = {}

# Feature toggles for hw bisection (all-on is the intended design).
import os as _os
FEAT = {
    "warmup": _os.environ.get("K_WARMUP", "1") == "1",
    "gp": _os.environ.get("K_GP", "1") == "1",       # gpsimd S-mult/tree
    "ttr": _os.environ.get("K_TTR", "1") == "1",     # fused qraw reduce
    "zmm": _os.environ.get("K_ZMM", "1") == "1",     # F=1 Z-column matmuls
}


def _pack(a):
    """(R, F) host array -> (128, R//128 * F) partition-major bf16."""
    a = np.asarray(a, dtype=np.float32)
    r, f = a.shape
    k = r // 128
    return np.ascontiguousarray(
        a.reshape(k, 128, f).transpose(1, 0, 2).reshape(128, k * f)
    ).astype(ml_dtypes.bfloat16)


def _build(has_gb_x, has_gb_y, has_hb, has_tb, _feat=None):
    nc = bacc.Bacc("TRN2", target_bir_lowering=False, debug=False)
    AF = mybir.ActivationFunctionType
    MULT = mybir.AluOpType.mult
    ADD = mybir.AluOpType.add

    def mm(out, lhsT, rhs, start, stop):
        nc.tensor.matmul(out, lhsT, rhs, start=start, stop=stop)

    def mmdr(out, lhsT, rhs, start, stop):
        nc.tensor.matmul(out, lhsT, rhs, start=start, stop=stop,
                         perf_mode=mybir.MatmulPerfMode.DoubleRow)

    # inputs host-packed to (BPC, 128, KC*N) partition-major
    d_sar = nc.dram_tensor("sar", [BPC, 128, KC * N], BF16, kind="ExternalInput")
    d_opt = nc.dram_tensor("opt", [BPC, 128, KC * N], BF16, kind="ExternalInput")
    d_sar8 = nc.dram_tensor("sar8", [BPC, 128, KC * N], FP8, kind="ExternalInput")
    d_opt8 = nc.dram_tensor("opt8", [BPC, 128, KC * N], FP8, kind="ExternalInput")
    # host-pretransposed + packed projection weights
    d_w = {}
    for nm in ("wt_tx", "wt_px", "wt_ty", "wt_py"):
        d_w[nm] = nc.dram_tensor(nm, [128, KC * CI], FP8, kind="ExternalInput")
    for nm in ("wt_gx", "wt_gy"):  # g weights carry a gamma/C ones column
        d_w[nm] = nc.dram_tensor(nm, [128, KC * (CI + 1)], BF16,
                                 kind="ExternalInput")
    d_hwT = nc.dram_tensor("hwT", [128, MC * HOUT], BF16, kind="ExternalInput")
    d_tb = {}
    if has_tb:  # theta/phi per-partition bias columns (CI,), fp32 (ACT bias)
        d_tb = {
            nm: nc.dram_tensor(nm, [CI], F32, kind="ExternalInput")
            for nm in ("b_tx", "b_px", "b_ty", "b_py")
        }
    d_ones = nc.dram_tensor("ones", [128, 256], BF16, kind="ExternalInput")
    need_onesr = has_gb_x or has_gb_y or has_hb
    if need_onesr:
        d_onesr = nc.dram_tensor("ones_row", [1, 128], BF16, kind="ExternalInput")
    d_expb = nc.dram_tensor("expb", [128, 1], F32, kind="ExternalInput")
    d_gb = {}
    if has_gb_x:
        d_gb["x"] = nc.dram_tensor("gb_x", [1, CI], BF16, kind="ExternalInput")
    if has_gb_y:
        d_gb["y"] = nc.dram_tensor("gb_y", [1, CI], BF16, kind="ExternalInput")
    if has_hb:
        d_hb = nc.dram_tensor("hb", [1, HOUT], BF16, kind="ExternalInput")
    d_out = nc.dram_tensor("out", [BPC, HOUT], F32, kind="ExternalOutput")

    with tile.TileContext(nc) as tc, \
            tc.tile_pool(name="wts", bufs=1) as wts, \
            tc.tile_pool(name="inp", bufs=1) as inp, \
            tc.tile_pool(name="proj", bufs=2) as proj, \
            tc.tile_pool(name="gp", bufs=2) as gpool, \
            tc.tile_pool(name="att", bufs=2) as attp, \
            tc.tile_pool(name="scr", bufs=3) as scr, \
            tc.tile_pool(name="rows", bufs=1) as rows, \
            tc.tile_pool(name="psA", bufs=2, space="PSUM") as psA, \
            tc.tile_pool(name="psG", bufs=2, space="PSUM") as psG, \
            tc.tile_pool(name="psU", bufs=2, space="PSUM") as psU:

        # ---- DMAs in strict first-use order (queues are FIFO) ----
        ones = wts.tile([128, 256], BF16, tag="ones", name="ones")
        nc.sync.dma_start(ones[:], d_ones.ap())

        def load_w(nm, cols=CI, dt_=None):
            t = wts.tile([128, KC, cols],
                         dt_ or (FP8 if nm[3] in "tp" else BF16), tag=nm, name=nm)
            nc.sync.dma_start(t[:], d_w[nm].ap().rearrange("p (k f) -> p k f", k=KC))
            return t

        w_sb = {}
        # first weight + first input gate the kernel start
        t = wts.tile([128, KC, CI], FP8, tag="wt_tx", name="wt_tx")
        w_sb["wt_tx"] = t
        nc.sync.dma_start(t[:, 0:2, :],
                          d_w["wt_tx"].ap()[:, :2 * CI].rearrange(
                              "p (k f) -> p k f", k=2))
        x8 = inp.tile([128, BPC, KC, N], FP8, tag="x8", name="x8")
        y8 = inp.tile([128, BPC, KC, N], FP8, tag="y8", name="y8")
        x_t = inp.tile([128, BPC, KC, N], BF16, tag="x_t", name="x_t")
        y_t = inp.tile([128, BPC, KC, N], BF16, tag="y_t", name="y_t")
        nc.sync.dma_start(x8[:, 0], d_sar8[0].rearrange("p (k n) -> p k n", k=KC))
        nc.sync.dma_start(t[:, 2:, :],
                          d_w["wt_tx"].ap()[:, 2 * CI:].rearrange(
                              "p (k f) -> p k f", k=KC - 2))
        w_sb["wt_px"] = load_w("wt_px")
        nc.sync.dma_start(x_t[:, 0], d_sar[0].rearrange("p (k n) -> p k n", k=KC))
        w_sb["wt_gx"] = load_w("wt_gx", CI + 1)
        w_sb["wt_ty"] = load_w("wt_ty")
        w_sb["wt_py"] = load_w("wt_py")
        nc.sync.dma_start(y8[:, 0], d_opt8[0].rearrange("p (k n) -> p k n", k=KC))
        w_sb["wt_gy"] = load_w("wt_gy", CI + 1)
        nc.sync.dma_start(y_t[:, 0], d_opt[0].rearrange("p (k n) -> p k n", k=KC))
        expb = wts.tile([128, 1], F32, tag="expb", name="expb")
        nc.sync.dma_start(expb[:], d_expb.ap())
        tb_sb = {}
        for nm, d in d_tb.items():
            tt = wts.tile([128, CIC], F32, tag=nm, name=nm)
            nc.sync.dma_start(tt[:], d.ap().rearrange("(k p) -> p k", p=128))
            tb_sb[nm] = tt
        for s in range(1, BPC):
            nc.sync.dma_start(x8[:, s], d_sar8[s].rearrange("p (k n) -> p k n", k=KC))
            nc.sync.dma_start(y8[:, s], d_opt8[s].rearrange("p (k n) -> p k n", k=KC))
            nc.sync.dma_start(x_t[:, s], d_sar[s].rearrange("p (k n) -> p k n", k=KC))
            nc.sync.dma_start(y_t[:, s], d_opt[s].rearrange("p (k n) -> p k n", k=KC))
        hwT = wts.tile([128, MC, HOUT], BF16, tag="hwT", name="hwT")
        nc.sync.dma_start(hwT[:], d_hwT.ap().rearrange("p (k f) -> p k f", k=MC))
        if need_onesr:
            ones_row = wts.tile([1, 128], BF16, tag="ones_row", name="ones_row")
            nc.sync.dma_start(ones_row[:], d_onesr.ap())
        gb_sb = {}
        for st, d in d_gb.items():
            tt = wts.tile([1, CI], BF16, tag=f"gb_{st}", name=f"gb_{st}")
            nc.sync.dma_start(tt[:], d.ap())
            gb_sb[st] = tt
        if has_hb:
            hb = wts.tile([1, HOUT], BF16, tag="hb", name="hb")
            nc.sync.dma_start(hb[:], d_hb.ap())

        in8 = {"x": x8, "y": y8}
        int_ = {"x": x_t, "y": y_t}
        pooledT = rows.tile([128, MC, BPC], BF16, tag="pooledT", name="pooledT")

        # ---- PE warm-up: ramp the pstate while input DMAs land ----
        for i in range(18 if FEAT["warmup"] else 0):
            pw = psU.tile([128, 512], F32, tag="u", name="pw")
            mm(pw[:, 0:256], ones[:, 0:128], ones[:], True, True)

        # PSUM-drain copy with round-robin engine assignment.
        # GpSimd cannot access PSUM, so drains rotate over ACT/DVE only.
        def copy_out(dst, src, eng):
            if eng == "act":
                nc.scalar.copy(dst, src)
            else:
                nc.vector.tensor_copy(dst, src)

        def emit_tp_chunk(s, ch, defer):
            """One theta/phi projection chunk (128 ci-rows x N) of sample s.
            Returns (dst, cic, pt) when the PSUM drain is deferred."""
            st, pr, cic, dst, eng = ch
            w = w_sb[f"wt_{pr}{st}"]
            pt = psA.tile([128, N], F32, tag="ps", name="ps")
            for kp in range(KC // 2):
                for o, f in NH:
                    mmdr(pt[:, o:o + f],
                         w[:, 2 * kp:2 * kp + 2, cic * 128:(cic + 1) * 128],
                         in8[st][:, s, 2 * kp:2 * kp + 2, o:o + f],
                         kp == 0, kp == KC // 2 - 1)
            if has_tb:
                nc.scalar.activation(dst[:, cic, :], pt[:], AF.Identity,
                                     bias=tb_sb[f"b_{pr}{st}"][:, cic:cic + 1])
                return None
            if defer:
                return (dst, cic, pt)
            copy_out(dst[:, cic, :], pt[:], eng)
            return None

        def tp_list(s, engs):
            pj = {}
            lst = []
            for i, (st, pr) in enumerate(
                    ((st, pr) for st in ("x", "y") for pr in ("t", "p"))):
                dst = proj.tile([128, CIC, N], FP8, tag=f"pj_{pr}{st}",
                                name=f"pj_{pr}{st}")
                pj[pr + st] = dst
                for cic in range(CIC):
                    lst.append((st, pr, cic, dst, engs[(2 * i + cic) % len(engs)]))
            return pj, lst

        def emit_g_chunk(s, sti, st, mc_, gXY, rscol, eng):
            """One g-projection column chunk; residual colsum rides col CI."""
            w = w_sb[f"wt_g{st}"]
            has_b = st in gb_sb
            pt = psG.tile([128, 512], F32, tag="g", name="gps")
            for k in range(KC):
                mm(pt[:, 0:CI + 1],
                   int_[st][:, s, k, mc_ * 128:(mc_ + 1) * 128],
                   w[:, k, :], k == 0, (k == KC - 1) and not has_b)
            if has_b:
                mm(pt[:, 0:CI], ones_row[:], gb_sb[st][:], False, True)
            copy_out(gXY[:, mc_, sti * CI:(sti + 1) * CI], pt[:, 0:CI], eng)
            if st == "x":
                nc.scalar.copy(rscol[:, mc_:mc_ + 1], pt[:, CI:CI + 1])
            else:
                nc.vector.tensor_add(rscol[:, mc_:mc_ + 1],
                                     rscol[:, mc_:mc_ + 1], pt[:, CI:CI + 1])

        def new_g_tiles():
            gXY = gpool.tile([128, MC, 2 * CI], BF16, tag="gXY", name="gXY")
            rscol = scr.tile([128, MC], F32, tag="rscol", name="rscol")
            return gXY, rscol

        # ---- preamble: theta/phi of sample 0 (copies split ACT/DVE) ----
        pj, lst0 = tp_list(0, ("act", "dve"))
        for ch in lst0:
            emit_tp_chunk(0, ch, False)

        for s in range(BPC):
            nxt = s + 1 < BPC
            # fillers interleaved into the logits phase: g(s) first (it gates
            # this iteration's U), mc-major so early chunks land before the
            # laddered U needs them, then theta/phi(s+1) whose last two PSUM
            # drains are deferred (on ACT) into the U phase.
            fillers = []
            if s == 0 or s == BPC - 1:
                gXY, rscol = new_g_tiles()
                for mc_ in range(MC):
                    for sti, st in enumerate(("x", "y")):
                        fillers.append(("g", (s, sti, st, mc_, gXY, rscol)))
            if nxt:
                pj_next, tpl = tp_list(s + 1, ("dve",))
                fillers += [("tp", ch) for ch in tpl]
            else:
                pj_next = None
            nf = len(fillers)

            # -- transposed logits + exp + fused map, fillers interleaved --
            E = {st: attp.tile([128, MC, N], BF16, tag=f"E{st}", name=f"E{st}")
                 for st in ("x", "y")}
            S = attp.tile([128, MC, N], BF16, tag="S", name="S")
            deferred = []
            fi = 0
            for mc_ in range(MC):
                for st in ("x", "y"):
                    pt = psA.tile([128, N], F32, tag="ps", name="ps")
                    for o, f in NH:
                        mmdr(pt[:, o:o + f],
                             pj["p" + st][:, :, mc_ * 128:(mc_ + 1) * 128],
                             pj["t" + st][:, :, o:o + f], True, True)
                    nc.scalar.activation(E[st][:, mc_, :], pt[:], AF.Exp,
                                         bias=expb[:])
                # fused map on SBUF operands: early chunks on idle GpSimd
                # (latency hidden behind later exps), tail chunks on DVE;
                # the last sample is tail-latency-critical: all DVE
                seng = nc.gpsimd if (mc_ < 3 and FEAT["gp"] and nxt) \
                    else nc.vector
                seng.tensor_mul(S[:, mc_, :], E["x"][:, mc_, :],
                                E["y"][:, mc_, :])
                hi = nf * (mc_ + 1) // MC
                while fi < hi:
                    kind, data = fillers[fi]
                    fi += 1
                    if kind == "g":
                        emit_g_chunk(*data, eng="dve")
                    else:
                        d = emit_tp_chunk(s + 1, data, fi > nf - 2)
                        if d is not None:
                            deferred.append(d)

            # -- deferred theta/phi drains (ACT has slack in the U phase) --
            for dst, cic, ptd in deferred:
                nc.scalar.copy(dst[:, cic, :], ptd[:])

            # -- Esum trees, pairwise so early adds overlap the exp phase
            # and the post-E5 critical path is two adds. Early pairs ride the
            # idle GpSimd except on the tail-critical last sample. --
            Es = {}
            geng = nc.gpsimd if (FEAT["gp"] and nxt) else nc.vector
            for st in ("x", "y"):
                P01 = scr.tile([128, N], BF16, tag="Pa", name="Pa")
                geng.tensor_add(P01[:], E[st][:, 0, :], E[st][:, 1, :])
                P23 = scr.tile([128, N], BF16, tag="Pb", name="Pb")
                nc.vector.tensor_add(P23[:], E[st][:, 2, :], E[st][:, 3, :])
                P45 = scr.tile([128, N], BF16, tag="Pc", name="Pc")
                nc.vector.tensor_add(P45[:], E[st][:, 4, :], E[st][:, 5, :])
                Q = scr.tile([128, N], BF16, tag="Q", name="Q")
                nc.vector.tensor_add(Q[:], P01[:], P23[:])
                Et = scr.tile([128, N], BF16, tag=f"Es{st}", name=f"Es{st}")
                nc.vector.tensor_add(Et[:], Q[:], P45[:])
                Es[st] = Et

            # -- attention apply, both streams per instruction; fused qraw --
            # Last sample has no theta/phi filler to absorb the exp drain, so
            # ladder the accumulation (2 live PSUM tiles) to start U earlier.
            qcol = scr.tile([128, MC], F32, tag="qcol", name="qcol")
            if s == BPC - 1:
                seq = []
                for n in range(MC + 1):
                    if n < MC:
                        seq.append((n, 0))
                    if n >= 1 and n < MC:
                        seq.append((n - 1, 1))
                    if n >= 1:
                        seq.append((n - 1, 2))
            else:
                seq = [(n, h) for n in range(MC) for h in (0, 1, 2)]
            pus = {}
            for nc_, third in seq:
                if third == 0:
                    pus[nc_] = psU.tile([128, 512], F32, tag="u", name="pu")
                pu = pus[nc_]
                for mc_ in range(2 * third, 2 * third + 2):
                    mm(pu[:], S[:, mc_, nc_ * 128:(nc_ + 1) * 128],
                       gXY[:, mc_, :], mc_ == 0, mc_ == MC - 1)
                if third == 2:
                    ux = scr.tile([128, CI], BF16, tag="ux", name="ux")
                    nc.scalar.copy(ux[:], pu[:, 0:CI])
                    yv = scr.tile([128, CI], BF16, tag="yv", name="yv")
                    nc.vector.tensor_mul(yv[:], ux[:], pu[:, CI:2 * CI])
                    nc.vector.tensor_reduce(
                        qcol[:, nc_:nc_ + 1], yv[:],
                        axis=mybir.AxisListType.X, op=ADD)
                    del pus[nc_]  # noqa

            # -- Z columns: F=1 matmuls on Esum chunks --
            zs = scr.tile([128, 12], F32, tag="zs", name="zs")
            if FEAT["zmm"]:
                zt = psU.tile([128, 512], F32, tag="u", name="zt")
                for sti, st in enumerate(("x", "y")):
                    for nc_ in range(MC):
                        mm(zt[:, 6 * sti + nc_:6 * sti + nc_ + 1],
                           Es[st][:, nc_ * 128:(nc_ + 1) * 128],
                           ones[:, 0:1], True, True)
                nc.scalar.copy(zs[:], zt[:, 0:12])
            else:
                nc.vector.memset(zs[:], 1.0)

            # -- column-space fixup: pooled = qraw/(Zx*Zy)^2 + rs --
            p1 = scr.tile([128, MC], F32, tag="p1", name="p1")
            nc.vector.tensor_mul(p1[:], zs[:, 0:6], zs[:, 6:12])
            p2 = scr.tile([128, MC], F32, tag="p2", name="p2")
            nc.vector.reciprocal(p2[:], p1[:])
            p3 = scr.tile([128, MC], F32, tag="p3", name="p3")
            nc.vector.tensor_mul(p3[:], p2[:], p2[:])
            p4 = scr.tile([128, MC], F32, tag="p4", name="p4")
            nc.vector.tensor_mul(p4[:], p3[:], qcol[:])
            nc.vector.tensor_add(pooledT[:, :, s], p4[:], rscol[:])

            # -- g projections of next sample (middle iterations only; the
            # last sample's g is its own phase-1 filler) --
            if nxt and s + 1 < BPC - 1:
                gXY, rscol = new_g_tiles()
                for sti, st in enumerate(("x", "y")):
                    for mc_ in range(MC):
                        emit_g_chunk(s + 1, sti, st, mc_, gXY, rscol, "dve")
            pj = pj_next

        # ---- head (samples 0..2 can start before the last fixup) ----
        pt = psU.tile([128, 512], F32, tag="u", name="head_ps")
        if has_hb:
            for j in range(MC):
                mm(pt[0:BPC, 0:HOUT], pooledT[:, j, :], hwT[:, j, :],
                   j == 0, False)
            mm(pt[0:BPC, 0:HOUT], ones_row[:, :BPC], hb[:], False, True)
        else:
            for j in range(MC):
                mm(pt[0:BPC - 1, 0:HOUT], pooledT[:, j, 0:BPC - 1],
                   hwT[:, j, :], j == 0, j == MC - 1)
            for j in range(MC):
                mm(pt[0:1, HOUT:2 * HOUT], pooledT[:, j, BPC - 1:BPC],
                   hwT[:, j, :], j == 0, j == MC - 1)
        if has_hb:
            out_sb = rows.tile([BPC, HOUT], F32, tag="out_sb", name="out_sb")
            nc.scalar.copy(out_sb[:], pt[0:BPC, 0:HOUT])
            nc.sync.dma_start(d_out[:], out_sb[:])
        else:
            out_a = rows.tile([BPC - 1, HOUT], F32, tag="out_a", name="out_a")
            nc.scalar.copy(out_a[:], pt[0:BPC - 1, 0:HOUT])
            out_b = rows.tile([1, HOUT], F32, tag="out_b", name="out_b")
            nc.scalar.copy(out_b[:], pt[0:1, HOUT:2 * HOUT])
            nc.sync.dma_start(d_out[0:BPC - 1], out_a[:])
            nc.sync.dma_start(d_out[BPC - 1:BPC], out_b[:])

    nc.compile()
    return nc


def _prepare(inputs):
    f = lambda k: np.ascontiguousarray(np.asarray(inputs[k], dtype=np.float32))
    bf = lambda a: np.ascontiguousarray(np.asarray(a, dtype=ml_dtypes.bfloat16))
    sar, opt = f("sar"), f("opt")
    ga = float(np.asarray(inputs["gamma_att"]).reshape(-1)[0])
    go = float(np.asarray(inputs["gamma_opt"]).reshape(-1)[0])
    gs = float(np.asarray(inputs["gamma_sar"]).reshape(-1)[0])
    W_w, W_b = f("W_w"), f("W_b")
    head_w, head_b = f("head_w"), f("head_b")

    wbar = (ga / C) * W_w.sum(axis=0)  # (CI,)
    bbar = (ga / C) * float(W_b.sum())
    # fold the pooled-constant through the head: out += bbar * head_w.sum(1)
    hb_eff = head_b + bbar * head_w.sum(axis=1)  # (HOUT,)

    gb_x = f("g_sar_b")
    gb_y = f("g_opt_b") * wbar  # wbar folded into the OPT value stream
    has_gb_x = bool(np.any(gb_x))
    has_gb_y = bool(np.any(gb_y))
    has_hb = bool(np.any(hb_eff))
    has_tb = any(bool(np.any(f(nm))) for nm in
                 ("theta_sar_b", "phi_sar_b", "theta_opt_b", "phi_opt_b"))

    key = (has_gb_x, has_gb_y, has_hb, has_tb, tuple(sorted(FEAT.items())))
    if key not in _cached:
        _cached[key] = _build(*key[:4])
    nc = _cached[key]

    # pack inputs: (B, C, N) -> per-core (BPC, 128, KC*N) partition-major
    def pack_in(a):
        a = a.reshape(B, KC, 128, N).transpose(0, 2, 1, 3).reshape(B, 128, KC * N)
        return np.ascontiguousarray(a).astype(ml_dtypes.bfloat16)

    sar_p, opt_p = pack_in(sar), pack_in(opt)

    p8 = lambda a: _pack(a).astype(ml_dtypes.float8_e4m3fn)
    common = {
        "wt_tx": p8(f("theta_sar_w").T),
        "wt_px": p8(f("phi_sar_w").T),
        "wt_ty": p8(f("theta_opt_w").T),
        "wt_py": p8(f("phi_opt_w").T),
        "wt_gx": _pack(np.concatenate(
            [f("g_sar_w").T, np.full((C, 1), gs / C, np.float32)], axis=1)),
        "wt_gy": _pack(np.concatenate(
            [(f("g_opt_w") * wbar[:, None]).T,
             np.full((C, 1), go / C, np.float32)], axis=1)),
        "hwT": _pack(head_w.T),
        "ones": np.ones((128, 256), ml_dtypes.bfloat16),
        "expb": np.full((128, 1), EXP_SHIFT, np.float32),
    }
    if has_tb:
        common.update({
            "b_tx": f("theta_sar_b"), "b_px": f("phi_sar_b"),
            "b_ty": f("theta_opt_b"), "b_py": f("phi_opt_b"),
        })
    if has_gb_x or has_gb_y or has_hb:
        common["ones_row"] = np.ones((1, 128), ml_dtypes.bfloat16)
    if has_gb_x:
        common["gb_x"] = bf(gb_x.reshape(1, CI))
    if has_gb_y:
        common["gb_y"] = bf(gb_y.reshape(1, CI))
    if has_hb:
        common["hb"] = bf(hb_eff.reshape(1, HOUT))

    in_maps = []
    for c in range(NCORES):
        m = dict(common)
        m["sar"] = np.ascontiguousarray(sar_p[c * BPC:(c + 1) * BPC])
        m["opt"] = np.ascontiguousarray(opt_p[c * BPC:(c + 1) * BPC])
        m["sar8"] = m["sar"].astype(ml_dtypes.float8_e4m3fn)
        m["opt8"] = m["opt"].astype(ml_dtypes.float8_e4m3fn)
        in_maps.append(m)
    return nc, in_maps


def kernel(**inputs):
    nc, in_maps = _prepare(inputs)
    res = run_bass_kernel_spmd(nc, in_maps, core_ids=list(range(NCORES)))
    return np.concatenate([res.results[c]["out"] for c in range(NCORES)], axis=0)


if __name__ == "__main__":
    rng = np.random.default_rng(0)
    ins = {
        "sar": rng.standard_normal((B, C, N), dtype=np.float32),
        "opt": rng.standard_normal((B, C, N), dtype=np.float32),
    }
    for nm in ("g_sar", "g_opt", "theta_sar", "theta_opt", "phi_sar", "phi_opt"):
        ins[nm + "_w"] = 0.02 * rng.standard_normal((CI, C), dtype=np.float32)
        ins[nm + "_b"] = np.zeros((CI,), np.float32)
    ins["W_w"] = 0.02 * rng.standard_normal((C, CI), dtype=np.float32)
    ins["W_b"] = np.zeros((C,), np.float32)
    ins["head_w"] = 0.02 * rng.standard_normal((HOUT, N), dtype=np.float32)
    ins["head_b"] = np.zeros((HOUT,), np.float32)
    ins["gamma_sar"] = np.asarray([0.3], np.float32)
    ins["gamma_opt"] = np.asarray([1.0], np.float32)
    ins["gamma_att"] = np.asarray([1.0], np.float32)
    out = kernel(**ins)
    print(out.shape, out.dtype, np.abs(out).mean())
